# Initial kernel scaffold
#
"""Trainium2 Bass kernel for a 4-layer post-norm dense transformer.

Reference math (per layer, post-norm):
    q/k/v = x @ w? + b?          (x is the raw residual stream)
    attn  = softmax(q k^T / 8) v ; attn_out = attn @ wo + bo
    x     = LN1(x + attn_out)
    ffn   = gelu_tanh(x @ w1 + b1) @ w2 + b2
    x     = LN2(x + ffn)

Sharding: sequence-parallel over 8 cores (256 rows each). Per layer each
core computes its K/V shard, AllGathers K then V (bf16), and everything
else is row-local. Activations keep features on partitions (x^T layout) so
every matmul contracts over partitions. Scores are computed transposed
([keys, queries]) so the softmax denominator comes from a ones-column
appended to V in the PV matmul, and exp needs no max-subtraction (post-LN
activations x 0.02-scale weights keep |scores| < ~2).

Residual-stream tensors are split into per-feature-chunk tiles so Tile's
per-tile dependency tracking lets next-phase matmuls start as soon as the
chunk they contract over is ready (LN overlaps compute).
"""

import numpy as np
import ml_dtypes

import concourse.bass as bass
import concourse.mybir as mybir
import concourse.tile as tile
from concourse import bacc, bass_utils
from concourse.bass import ds, ts

F32 = mybir.dt.float32
BF16 = mybir.dt.bfloat16
F8 = mybir.dt.float8e4
AF = mybir.ActivationFunctionType
OP = mybir.AluOpType

P = 128
D = 1024
H = 16
DK = 64
F = 4096
S = 2048
R = 8           # cores
SL = S // R     # 256 rows per core
FC = D // P     # 8 feature chunks
FFC = F // P    # 32 ffn feature chunks
KC = S // P     # 16 key chunks globally
EPS = 1e-5

_cache = {}


def build(n_layers=4):
    nc = bacc.Bacc("TRN2", target_bir_lowering=False, debug=False, num_devices=R)
    L = n_layers

    xT_d = nc.dram_tensor("xT", [D, SL], F32, kind="ExternalInput").ap()
    w_d = {}
    # pre-chunked on host: [L, n_chunks, 128 partitions(contraction), kc, m]
    for name, sh in [("wq", [L, 2, P, FC, 512]), ("wk", [L, 2, P, FC, 512]),
                     ("wv", [L, 2, P, FC, 512]), ("wo", [L, 2, P, FC, 512]),
                     ("w1", [L, 8, P, FC, 512]), ("w2", [L, 8, P, FFC, P])]:
        w_d[name] = nc.dram_tensor(name, sh, BF16, kind="ExternalInput").ap()
    b_d = {}
    for name, n in [("bq", D), ("bk", D), ("bo", D), ("b1", F), ("b2", D),
                    ("g1", D), ("be1", D), ("g2", D), ("be2", D)]:
        b_d[name] = nc.dram_tensor(name, [L, n], F32, kind="ExternalInput").ap()
    bv_d = nc.dram_tensor("bv", [L, D], BF16, kind="ExternalInput").ap()
    out_d = nc.dram_tensor("out", [D, SL], F32, kind="ExternalOutput").ap()

    with tile.TileContext(nc) as tc:
        _body(tc, nc, L, xT_d, w_d, b_d, bv_d, out_d)
    nc.compile()
    return nc


def _body(tc, nc, L, xT_d, w_d, b_d, bv_d, out_d):
    import contextlib
    ctx = contextlib.ExitStack()
    with ctx:
        sb = ctx.enter_context(tc.tile_pool(name="sb", bufs=1))
        sb2 = ctx.enter_context(tc.tile_pool(name="sb2", bufs=8))
        w8 = ctx.enter_context(tc.tile_pool(name="w8", bufs=4))
        w2p = ctx.enter_context(tc.tile_pool(name="w2p", bufs=4))
        prm = ctx.enter_context(tc.tile_pool(name="prm", bufs=2))
        scr = ctx.enter_context(tc.tile_pool(name="scr", bufs=2))
        dram = ctx.enter_context(tc.tile_pool(name="dram", bufs=2, space="DRAM"))
        pp = ctx.enter_context(tc.tile_pool(name="pp", bufs=2, space="PSUM"))
        ps = ctx.enter_context(tc.tile_pool(name="ps", bufs=2, space="PSUM"))
        pc = ctx.enter_context(tc.tile_pool(name="pc", bufs=2, space="PSUM"))

        # persistent state (per-feature-chunk tiles so deps are chunk-level)
        xb = [sb.tile([P, SL], BF16, tag=f"xb{i}", name=f"xb{i}") for i in range(FC)]
        KT = [sb.tile([P, FC, SL], BF16, tag=f"KT{r}", name=f"KT{r}") for r in range(R)]
        Vg = [sb.tile([P, 2, H * 65], F8, tag=f"Vg{r}", name=f"Vg{r}") for r in range(R)]
        hT = [sb.tile([P, 4, SL], BF16, tag=f"hT{i}", name=f"hT{i}") for i in range(FFC // 4)]
        ones_b = sb.tile([P, P], BF16, tag="ones_b")
        nc.vector.memset(ones_b[:], 1.0)
        Vg5 = [v[:].rearrange("p c (h e) -> p c h e", e=65) for v in Vg]
        for r in range(R):
            nc.vector.memset(Vg5[r][:, :, :, 64:65], 1.0)
        dumt = sb.tile([1, 16], F32, tag="dumt")

        def prefetch(func):
            # touch an ACT func so walrus's table load lands here (under PE
            # work) instead of on the next phase's critical path
            nc.scalar.activation(dumt[:], dumt[:], func)

        def warm(n):
            # no-dep LDWEIGHTS to keep the PE HAM clock at 8/8 through
            # phases where real matmuls trickle (exp-bound attention, LN)
            for _ in range(n):
                nc.tensor.ldweights(ones_b[:, 0:P])

        # initial residual load: x^T f32 -> bf16 cast (chunked)
        for fc in range(FC):
            xi = scr.tile([P, SL], F32, tag="t1", name=f"xi{fc}")
            nc.sync.dma_start(xi[:], xT_d.rearrange("(fc p) n -> p fc n", p=P)[:, fc, :])
            nc.vector.tensor_copy(out=xb[fc][:], in_=xi[:])

        for l in range(L):
            # ---- params for this layer ----
            bias = {}
            for nm in ("bq", "bk", "bo", "b2", "g1", "be1", "g2", "be2"):
                t = prm.tile([P, FC], F32, tag=nm)
                nc.sync.dma_start(t[:], b_d[nm][l].rearrange("(c p) -> p c", p=P))
                bias[nm] = t
            b1t = prm.tile([P, FFC], F32, tag="b1")
            nc.sync.dma_start(b1t[:], b_d["b1"][l].rearrange("(c p) -> p c", p=P))
            bvr = prm.tile([1, D], BF16, tag="bvr")
            nc.sync.dma_start(bvr[:], bv_d[l].rearrange("(a n) -> a n", a=1))

            # ---- K^T projection, ship, AllGather K ----
            KTloc = sb.tile([P, FC, SL], BF16, tag="KTloc")
            for half in range(2):
                wkh = w8.tile([P, FC, 512], BF16, tag="w8")
                nc.sync.dma_start(wkh[:], w_d["wk"][l, half])
                for mj in range(4):
                    mc = half * 4 + mj
                    psm = pp.tile([P, 512], F32, tag="pp")
                    for kc in range(FC):
                        nc.tensor.matmul(psm[:, :SL], wkh[:, kc, ts(mj, P)],
                                         xb[kc][:], start=(kc == 0), stop=(kc == FC - 1))
                    nc.vector.tensor_scalar_add(KTloc[:, mc, :], psm[:, :SL],
                                                bias["bk"][:, ds(mc, 1)])
            cc_in = dram.tile([D * SL], BF16, tag="cc_in")
            cc_out = dram.tile([R, D * SL], BF16, tag="cc_out", addr_space="Shared")
            ccv_in = dram.tile([D * SL], F8, tag="ccv_in")
            ccv_out = dram.tile([R, D * SL], F8, tag="ccv_out", addr_space="Shared")
            nc.sync.dma_start(cc_in[:].rearrange("(fc p n) -> p fc n", p=P, n=SL), KTloc[:])
            nc.gpsimd.collective_compute(
                "AllGather", OP.bypass, replica_groups=[list(range(R))],
                ins=[cc_in[:].opt()], outs=[cc_out[:].opt()])

            # ---- V projection (row layout) + bias, ship, AllGather V ----
            Vloc = sb.tile([P, 2, D], F8, tag="Vloc")
            bvb = ps.tile([P, 4, SL], F32, tag="ps")  # [128,1024] as 4x256
            for j in range(2):
                nc.tensor.matmul(bvb[:].rearrange("p a b -> p (a b)")[:, ds(j * 512, 512)],
                                 ones_b[0:1, :], bvr[:, ds(j * 512, 512)],
                                 start=True, stop=True)
            bvb_sb = prm.tile([P, D], BF16, tag="bvb_sb")
            nc.vector.tensor_copy(out=bvb_sb[:], in_=bvb[:].rearrange("p a b -> p (a b)"))
            for half in range(2):
                wvh = w8.tile([P, FC, 512], BF16, tag="w8")
                nc.sync.dma_start(wvh[:], w_d["wv"][l, half])
                for rc in range(2):
                    psm = pp.tile([P, 512], F32, tag="pp")
                    for kc in range(FC):
                        nc.tensor.matmul(psm[:], xb[kc][:, ts(rc, P)],
                                         wvh[:, kc, :], start=(kc == 0), stop=(kc == FC - 1))
                    nc.vector.tensor_tensor(
                        out=Vloc[:, rc, ds(half * 512, 512)], in0=psm[:],
                        in1=bvb_sb[:, ds(half * 512, 512)], op=OP.add)
            nc.sync.dma_start(ccv_in[:].rearrange("(rc p n) -> p rc n", p=P, n=D), Vloc[:])
            nc.gpsimd.collective_compute(
                "AllGather", OP.bypass, replica_groups=[list(range(R))],
                ins=[ccv_in[:].opt()], outs=[ccv_out[:].opt()])

            # ---- Q^T projection (overlaps collectives) ----
            QT = sb.tile([P, FC, SL], BF16, tag="QT")
            for half in range(2):
                wqh = w8.tile([P, FC, 512], BF16, tag="w8")
                nc.sync.dma_start(wqh[:], w_d["wq"][l, half])
                for mj in range(4):
                    mc = half * 4 + mj
                    psm = pp.tile([P, 512], F32, tag="pp")
                    for kc in range(FC):
                        nc.tensor.matmul(psm[:, :SL], wqh[:, kc, ts(mj, P)],
                                         xb[kc][:], start=(kc == 0), stop=(kc == FC - 1))
                    nc.vector.tensor_scalar_add(QT[:, mc, :], psm[:, :SL],
                                                bias["bq"][:, ds(mc, 1)])

            # ---- unpack gathered K/V ----
            for r in range(R):
                nc.sync.dma_start(
                    KT[r][:], cc_out[r].rearrange("(fc p n) -> p fc n", p=P, n=SL))
            for r in range(R):
                for rc2 in range(2):
                    nc.sync.dma_start(
                        Vg5[r][:, rc2, :, 0:64],
                        ccv_out[r].rearrange("(rc p h e) -> rc p h e", p=P, h=H, e=DK)[rc2])

            # ---- attention, head pairs interleaved (dense PE bursts) ----
            ctxN = [sb.tile([P, SL], BF16, tag=f"ctxN{i}", name=f"ctxN{i}") for i in range(FC)]
            for hp in range(H // 2):
                fh = hp
                cxe = pc.tile([65, SL], F32, tag="pc", name=f"cxe{hp}")
                cxo = pc.tile([65, SL], F32, tag="pc", name=f"cxo{hp}")
                for g in range(4):
                    sce = ps.tile([P, 4, SL], F32, tag="ps", name=f"sce{hp}_{g}")
                    sco = ps.tile([P, 4, SL], F32, tag="ps", name=f"sco{hp}_{g}")
                    for j in range(4):
                        kc = g * 4 + j
                        nc.tensor.matmul(sce[:, j, :],
                                         KT[kc // 2][0:64, fh, ts(kc % 2, P)],
                                         QT[0:64, fh, :], start=True, stop=True)
                        nc.tensor.matmul(sco[:, j, :],
                                         KT[kc // 2][64:128, fh, ts(kc % 2, P)],
                                         QT[64:128, fh, :], start=True, stop=True)
                    ae = sb2.tile([P, 4, SL], F8, tag="attn", name=f"ae{hp}_{g}")
                    ao = sb2.tile([P, 4, SL], F8, tag="attn", name=f"ao{hp}_{g}")
                    nc.scalar.activation(ae[:], sce[:], AF.Exp, scale=0.125)
                    nc.scalar.activation(ao[:], sco[:], AF.Exp, scale=0.125)
                    for i in range(2):
                        r = g * 2 + i
                        nc.tensor.matmul(cxe[:], Vg5[r][:, :, 2 * hp, :],
                                         ae[:, ds(2 * i, 2), :],
                                         perf_mode=mybir.MatmulPerfMode.DoubleRow,
                                         start=(r == 0), stop=(r == R - 1))
                        nc.tensor.matmul(cxo[:], Vg5[r][:, :, 2 * hp + 1, :],
                                         ao[:, ds(2 * i, 2), :],
                                         perf_mode=mybir.MatmulPerfMode.DoubleRow,
                                         start=(r == 0), stop=(r == R - 1))
                for e, cx in ((0, cxe), (1, cxo)):
                    p0 = e * 64
                    rcp = scr.tile([1, SL], F32, tag="rcp")
                    nc.vector.reciprocal(rcp[:], cx[64:65, :])
                    rcb = scr.tile([64, SL], F32, tag="rcb")
                    nc.gpsimd.partition_broadcast(rcb[:], rcp[:])
                    nc.vector.tensor_tensor(out=ctxN[fh][p0:p0 + 64, :], in0=cx[0:64, :],
                                            in1=rcb[:], op=OP.mult)
                if hp == H // 2 - 1:
                    prefetch(AF.Sqrt)

            # ---- out-proj + residual + LN1 ----
            x32 = [sb.tile([P, SL], F32, tag=f"x32_{i}", name=f"x32_{i}") for i in range(FC)]
            xpb = [sb.tile([P, SL], BF16, tag=f"xpb{i}", name=f"xpb{i}") for i in range(FC)]
            xsq = [sb.tile([P, SL], BF16, tag=f"xsq{i}", name=f"xsq{i}") for i in range(FC)]
            for half in range(2):
                woh = w8.tile([P, FC, 512], BF16, tag="w8")
                nc.sync.dma_start(woh[:], w_d["wo"][l, half])
                for mj in range(4):
                    mc = half * 4 + mj
                    psm = pp.tile([P, 512], F32, tag="pp")
                    for kc in range(FC):
                        nc.tensor.matmul(psm[:, :SL], woh[:, kc, ts(mj, P)],
                                         ctxN[kc][:], start=(kc == 0), stop=(kc == FC - 1))
                    nc.vector.scalar_tensor_tensor(
                        out=x32[mc][:], in0=psm[:, :SL],
                        scalar=bias["bo"][:, ds(mc, 1)], in1=xb[mc][:],
                        op0=OP.add, op1=OP.add)
                    nc.vector.tensor_copy(out=xpb[mc][:], in_=x32[mc][:])
                    nc.vector.tensor_tensor(out=xsq[mc][:], in0=xpb[mc][:],
                                            in1=xpb[mc][:], op=OP.mult)
            _layernorm(nc, pp, scr, sb, x32, xpb, xsq, ones_b,
                       bias["g1"], bias["be1"], xb, None)
            prefetch(AF.Gelu_apprx_tanh)

            # ---- FFN ----
            for mg in range(FFC // 4):
                w1h = w8.tile([P, FC, 512], BF16, tag="w8")
                nc.sync.dma_start(w1h[:], w_d["w1"][l, mg])
                hp = ps.tile([P, 4, SL], F32, tag="ps")
                for mj in range(4):
                    for kc in range(FC):
                        nc.tensor.matmul(hp[:, mj, :], w1h[:, kc, ts(mj, P)],
                                         xb[kc][:], start=(kc == 0), stop=(kc == FC - 1))
                    nc.scalar.activation(hT[mg][:, mj, :], hp[:, mj, :],
                                         AF.Gelu_apprx_tanh,
                                         bias=b1t[:, ds(mg * 4 + mj, 1)])
                if mg == FFC // 4 - 1:
                    prefetch(AF.Sqrt)
            last = (l == L - 1)
            for mc in range(FC):
                w2h = w2p.tile([P, FFC, P], BF16, tag="w2p")
                nc.sync.dma_start(w2h[:], w_d["w2"][l, mc])
                psm = pp.tile([P, 512], F32, tag="pp")
                for kc in range(FFC):
                    nc.tensor.matmul(psm[:, :SL], w2h[:, kc, :], hT[kc // 4][:, kc % 4, :],
                                     start=(kc == 0), stop=(kc == FFC - 1))
                nc.vector.scalar_tensor_tensor(
                    out=x32[mc][:], in0=psm[:, :SL],
                    scalar=bias["b2"][:, ds(mc, 1)], in1=xb[mc][:],
                    op0=OP.add, op1=OP.add)
                nc.vector.tensor_copy(out=xpb[mc][:], in_=x32[mc][:])
                nc.vector.tensor_tensor(out=xsq[mc][:], in0=xpb[mc][:],
                                        in1=xpb[mc][:], op=OP.mult)
            _layernorm(nc, pp, scr, sb, x32, xpb, xsq, ones_b,
                       bias["g2"], bias["be2"], xb, out_d if last else None)
            if not last:
                prefetch(AF.Exp)


def _layernorm(nc, pp, scr, pp2_sbc, x32, xpb, xsq, ones_b, g, b, xb_out, out_d):
    """x32/xpb/xsq: per-chunk lists ([128, SL] each); stats over features.
    Writes bf16 xb_out chunks, or f32 DMA to out_d if given (final layer)."""
    st = pp.tile([P, 512], F32, tag="pp")
    for fc in range(FC):
        nc.tensor.matmul(st[0:1, 0:SL], ones_b[:, 0:1], xpb[fc][:],
                         start=(fc == 0), stop=(fc == FC - 1))
    for fc in range(FC):
        nc.tensor.matmul(st[0:1, SL:2 * SL], ones_b[:, 0:1], xsq[fc][:],
                         start=(fc == 0), stop=(fc == FC - 1))
    s2 = scr.tile([1, 512], F32, tag="s2")
    v0 = scr.tile([1, SL], F32, tag="v0")
    # mu; musq - eps; var+eps = m2 - (musq - eps); 1/(var+eps); sqrt -> a
    nc.vector.tensor_scalar_mul(s2[:, 0:SL], st[0:1, 0:SL], 1.0 / D)
    nc.vector.tensor_tensor(out=v0[:], in0=s2[:, 0:SL], in1=s2[:, 0:SL], op=OP.mult)
    nc.vector.tensor_scalar_add(v0[:], v0[:], -EPS)
    nc.vector.scalar_tensor_tensor(out=v0[:], in0=st[0:1, SL:2 * SL],
                                   scalar=1.0 / D, in1=v0[:],
                                   op0=OP.mult, op1=OP.subtract)
    nc.vector.reciprocal(v0[:], v0[:])
    nc.scalar.activation(s2[:, SL:2 * SL], v0[:], AF.Sqrt)
    sbc = pp2_sbc.tile([P, 512], F32, tag="sbc")
    nc.gpsimd.partition_broadcast(sbc[:], s2[:])
    for fc in range(FC):
        t1 = scr.tile([P, SL], F32, tag="t1")
        nc.vector.tensor_tensor(out=t1[:], in0=x32[fc][:], in1=sbc[:, 0:SL],
                                op=OP.subtract)
        nc.vector.tensor_tensor(out=t1[:], in0=t1[:], in1=sbc[:, SL:2 * SL],
                                op=OP.mult)
        if out_d is None:
            nc.vector.tensor_scalar(xb_out[fc][:], t1[:],
                                    g[:, ds(fc, 1)], b[:, ds(fc, 1)],
                                    OP.mult, OP.add)
        else:
            o1 = scr.tile([P, SL], F32, tag="o1")
            nc.vector.tensor_scalar(o1[:], t1[:],
                                    g[:, ds(fc, 1)], b[:, ds(fc, 1)],
                                    OP.mult, OP.add)
            nc.sync.dma_start(
                out_d.rearrange("(fc p) n -> p fc n", p=P)[:, fc, :], o1[:])


def _prep_inputs(inputs, n_layers):
    """Full inputs -> per-core in_maps."""
    x = np.asarray(inputs["x"])          # [1, 2048, 1024] f32
    L = n_layers
    bf = ml_dtypes.bfloat16
    def chunk_w(w, m_chunk):
        # [L, K, M] -> [L, M//m_chunk, 128, K//128, m_chunk] matching SBUF lhsT tiles
        Lw, Kw, Mw = w.shape
        a = w.reshape(Lw, Kw // P, P, Mw // m_chunk, m_chunk)
        return np.ascontiguousarray(a.transpose(0, 3, 2, 1, 4)).astype(bf)

    base = {
        "wq": chunk_w(np.asarray(inputs["wq"][:L]), 512),
        "wk": chunk_w(np.asarray(inputs["wk"][:L]), 512),
        "wv": chunk_w(np.asarray(inputs["wv"][:L]), 512),
        "wo": chunk_w(np.asarray(inputs["wo"][:L]), 512),
        "w1": chunk_w(np.asarray(inputs["w1"][:L]), 512),
        "w2": chunk_w(np.asarray(inputs["w2"][:L]), P),
        "bq": np.ascontiguousarray(inputs["bq"][:L]).astype(np.float32),
        "bk": np.ascontiguousarray(inputs["bk"][:L]).astype(np.float32),
        "bo": np.ascontiguousarray(inputs["bo"][:L]).astype(np.float32),
        "b1": np.ascontiguousarray(inputs["b1"][:L]).astype(np.float32),
        "b2": np.ascontiguousarray(inputs["b2"][:L]).astype(np.float32),
        "bv": np.ascontiguousarray(inputs["bv"][:L]).astype(bf),
        "g1": np.ascontiguousarray(inputs["ln1_g"][:L]).astype(np.float32),
        "be1": np.ascontiguousarray(inputs["ln1_b"][:L]).astype(np.float32),
        "g2": np.ascontiguousarray(inputs["ln2_g"][:L]).astype(np.float32),
        "be2": np.ascontiguousarray(inputs["ln2_b"][:L]).astype(np.float32),
    }
    in_maps = []
    for c in range(R):
        m = dict(base)
        m["xT"] = np.ascontiguousarray(x[0, c * SL:(c + 1) * SL, :].T.astype(np.float32))
        in_maps.append(m)
    return in_maps


def run(inputs, n_layers=4, trace=False):
    if n_layers not in _cache:
        _cache[n_layers] = build(n_layers)
    nc = _cache[n_layers]
    in_maps = _prep_inputs(inputs, n_layers)
    res = bass_utils.run_bass_kernel_spmd(nc, in_maps, core_ids=list(range(R)),
                                          trace=trace)
    out = np.empty((1, S, D), dtype=np.float32)
    for c in range(R):
        out[0, c * SL:(c + 1) * SL, :] = res.results[c]["out"].T
    return out, res


def kernel(**inputs) -> np.ndarray:
    out, _ = run(inputs, n_layers=4, trace=False)
    return out



# revision 25
# speedup vs baseline: 1.2564x; 1.2564x over previous
"""Trainium2 Bass kernel for a 4-layer post-norm dense transformer.

Reference math (per layer, post-norm):
    q/k/v = x @ w? + b?          (x is the raw residual stream)
    attn  = softmax(q k^T / 8) v ; attn_out = attn @ wo + bo
    x     = LN1(x + attn_out)
    ffn   = gelu_tanh(x @ w1 + b1) @ w2 + b2
    x     = LN2(x + ffn)

Sharding: sequence-parallel over 8 cores (256 rows each). Per layer each
core computes its K/V shard, AllGathers K (fp8) then V (fp8), and
everything else is row-local. Activations keep features on partitions
(x^T layout) so every matmul contracts over partitions. Scores are
computed transposed ([keys, queries]) so the softmax denominator comes
from a ones-column appended to V in the PV matmul, and exp needs no
max-subtraction (post-LN activations x 0.02-scale weights keep
|scores| < ~3).

K and Q are quantized to fp8e4 (scores get ~1% extra noise, well inside
the error budget) which halves the K AllGather and enables fast weight
loads on the score matmuls. Collective staging buffers are laid out
partition-major on both sides so every ship/unpack DMA moves 2KB+
contiguous runs per partition. The attention loop is software-pipelined
(scores/exp run two head-pairs ahead of PV/normalize) so the PE keeps
working while the scalar engine chews through the exps and while the V
gather is still in flight.
"""

import numpy as np
import ml_dtypes

import concourse.bass as bass
import concourse.mybir as mybir
import concourse.tile as tile
from concourse import bacc, bass_utils
from concourse.bass import ds, ts

F32 = mybir.dt.float32
BF16 = mybir.dt.bfloat16
F8 = mybir.dt.float8e4
AF = mybir.ActivationFunctionType
OP = mybir.AluOpType

P = 128
D = 1024
H = 16
DK = 64
F = 4096
S = 2048
R = 8           # cores
SL = S // R     # 256 rows per core
FC = D // P     # 8 feature chunks
FFC = F // P    # 32 ffn feature chunks
KC = S // P     # 16 key chunks globally
EPS = 1e-5

_cache = {}


def build(n_layers=4):
    nc = bacc.Bacc("TRN2", target_bir_lowering=False, debug=False, num_devices=R)
    L = n_layers

    xT_d = nc.dram_tensor("xT", [D, SL], F32, kind="ExternalInput").ap()
    w_d = {}
    # pre-chunked on host: [L, n_chunks, 128 partitions(contraction), kc, m]
    for name, sh in [("wq", [L, 2, P, FC, 512]), ("wk", [L, 2, P, FC, 512]),
                     ("wv", [L, 2, P, FC, 512]), ("wo", [L, 2, P, FC, 512]),
                     ("w1", [L, 8, P, FC, 512]), ("w2", [L, 8, P, FFC, P])]:
        w_d[name] = nc.dram_tensor(name, sh, BF16, kind="ExternalInput").ap()
    b_d = {}
    for name, n in [("bq", D), ("bk", D), ("bo", D), ("b1", F), ("b2", D),
                    ("g1", D), ("be1", D), ("g2", D), ("be2", D)]:
        b_d[name] = nc.dram_tensor(name, [L, n], F32, kind="ExternalInput").ap()
    bv_d = nc.dram_tensor("bv", [L, D], BF16, kind="ExternalInput").ap()
    out_d = nc.dram_tensor("out", [D, SL], F32, kind="ExternalOutput").ap()

    with tile.TileContext(nc) as tc:
        _body(tc, nc, L, xT_d, w_d, b_d, bv_d, out_d)
    nc.compile()
    return nc


def _body(tc, nc, L, xT_d, w_d, b_d, bv_d, out_d):
    import contextlib
    ctx = contextlib.ExitStack()
    with ctx:
        sb = ctx.enter_context(tc.tile_pool(name="sb", bufs=1))
        sb2 = ctx.enter_context(tc.tile_pool(name="sb2", bufs=16))
        w8 = ctx.enter_context(tc.tile_pool(name="w8", bufs=4))
        w2p = ctx.enter_context(tc.tile_pool(name="w2p", bufs=4))
        prm = ctx.enter_context(tc.tile_pool(name="prm", bufs=2))
        scr = ctx.enter_context(tc.tile_pool(name="scr", bufs=2))
        dram = ctx.enter_context(tc.tile_pool(name="dram", bufs=2, space="DRAM"))
        pp = ctx.enter_context(tc.tile_pool(name="pp", bufs=2, space="PSUM"))
        ps = ctx.enter_context(tc.tile_pool(name="ps", bufs=2, space="PSUM"))
        pc = ctx.enter_context(tc.tile_pool(name="pc", bufs=2, space="PSUM"))

        # persistent state (per-feature-chunk tiles so deps are chunk-level)
        xb = [sb.tile([P, SL], BF16, tag=f"xb{i}", name=f"xb{i}") for i in range(FC)]
        KT = [sb.tile([P, FC, SL], F8, tag=f"KT{r}", name=f"KT{r}") for r in range(R)]
        Vg = [sb.tile([P, 2, H, 65], F8, tag=f"Vg{r}", name=f"Vg{r}") for r in range(R)]
        hT = [sb.tile([P, 4, SL], BF16, tag=f"hT{i}", name=f"hT{i}") for i in range(FFC // 4)]
        ones_b = sb.tile([P, P], BF16, tag="ones_b")
        nc.vector.memset(ones_b[:], 1.0)
        ones32 = sb.tile([1, P], F32, tag="ones32")
        nc.vector.memset(ones32[:], 1.0)
        # V ships with its ones-column (PV denominator trick) baked in
        Vloc = sb.tile([P, 2, H, 65], F8, tag="Vloc")
        nc.vector.memset(Vloc[:, :, :, 64:65], 1.0)
        dumt = sb.tile([1, 16], F32, tag="dumt")

        def prefetch(func):
            # touch an ACT func so walrus's table load lands here (under PE
            # work) instead of on the next phase's critical path
            nc.scalar.activation(dumt[:], dumt[:], func)

        # initial residual load: x^T f32 -> bf16 cast (chunked)
        for fc in range(FC):
            xi = scr.tile([P, SL], F32, tag="t1", name=f"xi{fc}")
            nc.sync.dma_start(xi[:], xT_d.rearrange("(fc p) n -> p fc n", p=P)[:, fc, :])
            nc.vector.tensor_copy(out=xb[fc][:], in_=xi[:])

        for l in range(L):
            # ---- params for this layer ----
            bias = {}
            for nm in ("bq", "bk", "bo", "b2", "g1", "be1", "g2", "be2"):
                t = prm.tile([P, FC], F32, tag=nm)
                nc.sync.dma_start(t[:], b_d[nm][l].rearrange("(c p) -> p c", p=P))
                bias[nm] = t
            b1t = prm.tile([P, FFC], F32, tag="b1")
            nc.sync.dma_start(b1t[:], b_d["b1"][l].rearrange("(c p) -> p c", p=P))
            bvr = prm.tile([1, D], BF16, tag="bvr")
            nc.sync.dma_start(bvr[:], bv_d[l].rearrange("(a n) -> a n", a=1))

            # ---- K^T projection (fp8 out), ship p-major, AllGather K ----
            KTloc = sb.tile([P, FC, SL], F8, tag="KTloc")
            for half in range(2):
                wkh = w8.tile([P, FC, 512], BF16, tag="w8")
                nc.sync.dma_start(wkh[:], w_d["wk"][l, half])
                for mj in range(4):
                    mc = half * 4 + mj
                    psm = pp.tile([P, 512], F32, tag="pp")
                    for kc in range(FC):
                        nc.tensor.matmul(psm[:, :SL], wkh[:, kc, ts(mj, P)],
                                         xb[kc][:], start=(kc == 0), stop=(kc == FC - 1))
                    nc.vector.tensor_scalar_add(KTloc[:, mc, :], psm[:, :SL],
                                                bias["bk"][:, ds(mc, 1)])
            ccK_in = dram.tile([D * SL], F8, tag="ccK_in")
            ccK_out = dram.tile([R, D * SL], F8, tag="ccK_out", addr_space="Shared")
            nc.sync.dma_start(
                ccK_in[:].rearrange("(p fc n) -> p fc n", p=P, n=SL), KTloc[:])
            nc.gpsimd.collective_compute(
                "AllGather", OP.bypass, replica_groups=[list(range(R))],
                ins=[ccK_in[:].opt()], outs=[ccK_out[:].opt()])

            # ---- V projection (row layout, holes for ones col) + AllGather V ----
            bvb = ps.tile([P, 2, 2, SL], F32, tag="ps")  # [128,1024] as 4x256
            for j in range(2):
                nc.tensor.matmul(bvb[:].rearrange("p a b c -> p (a b c)")[:, ds(j * 512, 512)],
                                 ones_b[0:1, :], bvr[:, ds(j * 512, 512)],
                                 start=True, stop=True)
            bvb_sb = prm.tile([P, D], BF16, tag="bvb_sb")
            nc.vector.tensor_copy(out=bvb_sb[:], in_=bvb[:].rearrange("p a b c -> p (a b c)"))
            for half in range(2):
                wvh = w8.tile([P, FC, 512], BF16, tag="w8")
                nc.sync.dma_start(wvh[:], w_d["wv"][l, half])
                for rc in range(2):
                    psm = pp.tile([P, 512], F32, tag="pp")
                    for kc in range(FC):
                        nc.tensor.matmul(psm[:], xb[kc][:, ts(rc, P)],
                                         wvh[:, kc, :], start=(kc == 0), stop=(kc == FC - 1))
                    nc.vector.tensor_tensor(
                        out=Vloc[:, rc, ds(half * 8, 8), 0:64],
                        in0=psm[:].rearrange("p (h e) -> p h e", e=64),
                        in1=bvb_sb[:, ds(half * 512, 512)].rearrange("p (h e) -> p h e", e=64),
                        op=OP.add)
            ccV_in = dram.tile([P * 2 * H * 65], F8, tag="ccV_in")
            ccV_out = dram.tile([R, P * 2 * H * 65], F8, tag="ccV_out", addr_space="Shared")
            nc.sync.dma_start(
                ccV_in[:].rearrange("(p c h e) -> p c h e", p=P, c=2, e=65), Vloc[:])
            nc.gpsimd.collective_compute(
                "AllGather", OP.bypass, replica_groups=[list(range(R))],
                ins=[ccV_in[:].opt()], outs=[ccV_out[:].opt()])

            # ---- Q^T projection (fp8, overlaps collectives) ----
            QT = sb.tile([P, FC, SL], F8, tag="QT")
            for half in range(2):
                wqh = w8.tile([P, FC, 512], BF16, tag="w8")
                nc.sync.dma_start(wqh[:], w_d["wq"][l, half])
                for mj in range(4):
                    mc = half * 4 + mj
                    psm = pp.tile([P, 512], F32, tag="pp")
                    for kc in range(FC):
                        nc.tensor.matmul(psm[:, :SL], wqh[:, kc, ts(mj, P)],
                                         xb[kc][:], start=(kc == 0), stop=(kc == FC - 1))
                    nc.vector.tensor_scalar_add(QT[:, mc, :], psm[:, :SL],
                                                bias["bq"][:, ds(mc, 1)])

            # ---- unpack gathered K/V (contiguous per partition) ----
            for r in range(R):
                nc.sync.dma_start(
                    KT[r][:], ccK_out[r].rearrange("(p fc n) -> p fc n", p=P, n=SL))
            for r in range(R):
                nc.sync.dma_start(
                    Vg[r][:], ccV_out[r].rearrange("(p c h e) -> p c h e",
                                                   p=P, c=2, e=65))

            # ---- weight prefetch: emit O-proj/FFN weight DMAs now so they
            # land during attention (their pool buffers are free). woh must
            # precede w1h: they share a pool+queue, and w1h(4+) can only run
            # after the FFN starts, which needs woh. ----
            # queue order matters: woh first (FFN depends on O-proj), then the
            # w1h/w2h whose buffers are free now (land during attention), then
            # the tails whose buffer-WAR waits resolve mid-FFN.
            wohs = []
            for half in range(2):
                woh = w8.tile([P, FC, 512], BF16, tag="w8", name=f"woh{half}")
                nc.sync.dma_start(woh[:], w_d["wo"][l, half])
                wohs.append(woh)
            w1hs = []
            for mg in range(4):
                w1h = w8.tile([P, FC, 512], BF16, tag="w8", name=f"w1h{mg}")
                nc.sync.dma_start(w1h[:], w_d["w1"][l, mg])
                w1hs.append(w1h)
            w2hs = []
            for mc in range(4):
                w2h = w2p.tile([P, FFC, P], BF16, tag="w2p", name=f"w2h{mc}")
                nc.sync.dma_start(w2h[:], w_d["w2"][l, mc])
                w2hs.append(w2h)
            for mg in range(4, FFC // 4):
                w1h = w8.tile([P, FC, 512], BF16, tag="w8", name=f"w1h{mg}")
                nc.sync.dma_start(w1h[:], w_d["w1"][l, mg])
                w1hs.append(w1h)
            for mc in range(4, FC):
                w2h = w2p.tile([P, FFC, P], BF16, tag="w2p", name=f"w2h{mc}")
                nc.sync.dma_start(w2h[:], w_d["w2"][l, mc])
                w2hs.append(w2h)

            # ---- attention: software-pipelined scores/exp vs PV/normalize ----
            # score tile g holds key block r=g (256 keys) for both heads of
            # the pair: [keys 128, parity 2, kc 2, q 256]; one exp per tile.
            ctxN = [sb.tile([P, SL], BF16, tag=f"ctxN{i}", name=f"ctxN{i}") for i in range(FC)]
            aes = {}   # (hp, g) -> fp8 prob tile

            def scores_g(hp, g):
                sc = ps.tile([P, 2, 2, SL], F32, tag="ps", name=f"sc{hp}_{g}")
                for j in range(2):
                    kc = g * 2 + j
                    nc.tensor.matmul(sc[:, 0, j, :],
                                     KT[g][0:64, hp, ts(j, P)],
                                     QT[0:64, hp, :], start=True, stop=True)
                    nc.tensor.matmul(sc[:, 1, j, :],
                                     KT[g][64:128, hp, ts(j, P)],
                                     QT[64:128, hp, :], start=True, stop=True)
                ae = sb2.tile([P, 2, 2, SL], F8, tag="attn", name=f"ae{hp}_{g}")
                nc.scalar.activation(ae[:], sc[:], AF.Exp, scale=0.125)
                aes[(hp, g)] = ae

            def pv_g(hp, g, cx):
                ae = aes.pop((hp, g))
                nc.tensor.matmul(cx[:, 0:SL], Vg[g][:, :, 2 * hp, :],
                                 ae[:, 0, :, :],
                                 perf_mode=mybir.MatmulPerfMode.DoubleRow,
                                 start=(g == 0), stop=(g == R - 1))
                nc.tensor.matmul(cx[:, SL:2 * SL], Vg[g][:, :, 2 * hp + 1, :],
                                 ae[:, 1, :, :],
                                 perf_mode=mybir.MatmulPerfMode.DoubleRow,
                                 start=(g == 0), stop=(g == R - 1))

            def epi(hp, cx):
                # evacuate PSUM fast (frees the bank for the next head-pair),
                # then normalize from the SBUF copy off the critical path
                cu = scr.tile([65, 2 * SL], F32, tag="ctxU", name=f"cu{hp}")
                nc.vector.tensor_copy(out=cu[:], in_=cx[:])
                rcp = scr.tile([1, 2 * SL], F32, tag="rcp")
                nc.vector.reciprocal(rcp[:], cu[64:65, :])
                rcb = scr.tile([64, 2 * SL], F32, tag="rcb")
                nc.gpsimd.partition_broadcast(rcb[:], rcp[:])
                for e in range(2):
                    nc.vector.tensor_tensor(out=ctxN[hp][e * 64:(e + 1) * 64, :],
                                            in0=cu[0:64, ds(e * SL, SL)],
                                            in1=rcb[:, ds(e * SL, SL)], op=OP.mult)

            DEPTH = 2
            cxs = {}
            for hp in range(H // 2 + DEPTH):
                if DEPTH <= hp:
                    cxs[hp - DEPTH] = pc.tile([65, 2 * SL], F32, tag="pc",
                                              name=f"cx{hp - DEPTH}")
                for g in range(R):
                    if DEPTH <= hp:
                        pv_g(hp - DEPTH, g, cxs[hp - DEPTH])
                    if hp < H // 2:
                        scores_g(hp, g)
                if DEPTH <= hp:
                    epi(hp - DEPTH, cxs.pop(hp - DEPTH))
                if hp == H // 2 - 1:
                    prefetch(AF.Sqrt)

            # ---- out-proj + residual + LN1 ----
            x32 = [sb.tile([P, SL], F32, tag=f"x32_{i}", name=f"x32_{i}") for i in range(FC)]
            xpb = [sb.tile([P, SL], BF16, tag=f"xpb{i}", name=f"xpb{i}") for i in range(FC)]
            xsq = [sb.tile([P, SL], BF16, tag=f"xsq{i}", name=f"xsq{i}") for i in range(FC)]
            for half in range(2):
                woh = wohs[half]
                for mj in range(4):
                    mc = half * 4 + mj
                    psm = pp.tile([P, 512], F32, tag="pp")
                    for kc in range(FC):
                        nc.tensor.matmul(psm[:, :SL], woh[:, kc, ts(mj, P)],
                                         ctxN[kc][:], start=(kc == 0), stop=(kc == FC - 1))
                    nc.vector.scalar_tensor_tensor(
                        out=x32[mc][:], in0=psm[:, :SL],
                        scalar=bias["bo"][:, ds(mc, 1)], in1=xb[mc][:],
                        op0=OP.add, op1=OP.add)
                    nc.vector.tensor_copy(out=xpb[mc][:], in_=x32[mc][:])
                    nc.vector.tensor_tensor(out=xsq[mc][:], in0=xpb[mc][:],
                                            in1=xpb[mc][:], op=OP.mult)
            _layernorm(nc, pp, scr, ones32, x32, xpb, xsq, ones_b,
                       bias["g1"], bias["be1"], xb, None)
            prefetch(AF.Gelu_apprx_tanh)

            # ---- FFN ----
            for mg in range(FFC // 4):
                w1h = w1hs[mg]
                hp_ = ps.tile([P, 2, 2, SL], F32, tag="ps")
                for mj in range(4):
                    for kc in range(FC):
                        nc.tensor.matmul(hp_[:, mj // 2, mj % 2, :],
                                         w1h[:, kc, ts(mj, P)],
                                         xb[kc][:], start=(kc == 0), stop=(kc == FC - 1))
                    nc.scalar.activation(hT[mg][:, mj, :], hp_[:, mj // 2, mj % 2, :],
                                         AF.Gelu_apprx_tanh,
                                         bias=b1t[:, ds(mg * 4 + mj, 1)])
                if mg == FFC // 4 - 1:
                    prefetch(AF.Sqrt)
            last = (l == L - 1)
            for mc in range(FC):
                w2h = w2hs[mc]
                psm = pp.tile([P, 512], F32, tag="pp")
                for kc in range(FFC):
                    nc.tensor.matmul(psm[:, :SL], w2h[:, kc, :], hT[kc // 4][:, kc % 4, :],
                                     start=(kc == 0), stop=(kc == FFC - 1))
                nc.vector.scalar_tensor_tensor(
                    out=x32[mc][:], in0=psm[:, :SL],
                    scalar=bias["b2"][:, ds(mc, 1)], in1=xb[mc][:],
                    op0=OP.add, op1=OP.add)
                nc.vector.tensor_copy(out=xpb[mc][:], in_=x32[mc][:])
                nc.vector.tensor_tensor(out=xsq[mc][:], in0=xpb[mc][:],
                                        in1=xpb[mc][:], op=OP.mult)
            _layernorm(nc, pp, scr, ones32, x32, xpb, xsq, ones_b,
                       bias["g2"], bias["be2"], xb, out_d if last else None)
            if not last:
                prefetch(AF.Exp)


def _layernorm(nc, pp, scr, ones32, x32, xpb, xsq, ones_b, g, b, xb_out, out_d):
    """x32/xpb/xsq: per-chunk lists ([128, SL] each); stats over features.
    Writes bf16 xb_out chunks, or f32 DMA to out_d if given (final layer)."""
    st = pp.tile([P, 512], F32, tag="pp")
    for fc in range(FC):
        nc.tensor.matmul(st[0:1, 0:SL], ones_b[:, 0:1], xpb[fc][:],
                         start=(fc == 0), stop=(fc == FC - 1))
    for fc in range(FC):
        nc.tensor.matmul(st[0:1, SL:2 * SL], ones_b[:, 0:1], xsq[fc][:],
                         start=(fc == 0), stop=(fc == FC - 1))
    s2 = scr.tile([1, 512], F32, tag="s2")
    v0 = scr.tile([1, SL], F32, tag="v0")
    v1 = scr.tile([1, SL], F32, tag="v1")
    # mu; musq - eps; var+eps = m2 - (musq - eps); 1/(var+eps); sqrt -> a
    nc.vector.tensor_scalar_mul(s2[:, 0:SL], st[0:1, 0:SL], 1.0 / D)
    # dummy sqrt tied to the early stats so its ACT table load runs during
    # the reciprocal instead of serializing after it
    dumt2 = scr.tile([1, 16], F32, tag="dumt2")
    nc.scalar.activation(dumt2[:], s2[:, 0:16], AF.Sqrt)
    nc.vector.tensor_tensor(out=v0[:], in0=s2[:, 0:SL], in1=s2[:, 0:SL], op=OP.mult)
    nc.vector.tensor_scalar_add(v0[:], v0[:], -EPS)
    nc.vector.scalar_tensor_tensor(out=v0[:], in0=st[0:1, SL:2 * SL],
                                   scalar=1.0 / D, in1=v0[:],
                                   op0=OP.mult, op1=OP.subtract)
    nc.vector.reciprocal(v1[:], v0[:])
    nc.scalar.activation(s2[:, SL:2 * SL], v1[:], AF.Sqrt)
    sbc = scr.tile([P, 512], F32, tag="sbc")
    nc.gpsimd.partition_broadcast(sbc[:], s2[:])
    for fc in range(FC):
        t1 = scr.tile([P, SL], F32, tag="t1")
        nc.vector.tensor_tensor(out=t1[:], in0=x32[fc][:], in1=sbc[:, 0:SL],
                                op=OP.subtract)
        nc.vector.tensor_tensor(out=t1[:], in0=t1[:], in1=sbc[:, SL:2 * SL],
                                op=OP.mult)
        if out_d is None:
            nc.vector.tensor_scalar(xb_out[fc][:], t1[:],
                                    g[:, ds(fc, 1)], b[:, ds(fc, 1)],
                                    OP.mult, OP.add)
        else:
            o1 = scr.tile([P, SL], F32, tag="o1")
            nc.vector.tensor_scalar(o1[:], t1[:],
                                    g[:, ds(fc, 1)], b[:, ds(fc, 1)],
                                    OP.mult, OP.add)
            nc.sync.dma_start(
                out_d.rearrange("(fc p) n -> p fc n", p=P)[:, fc, :], o1[:])


def _prep_inputs(inputs, n_layers):
    """Full inputs -> per-core in_maps."""
    x = np.asarray(inputs["x"])          # [1, 2048, 1024] f32
    L = n_layers
    bf = ml_dtypes.bfloat16
    def chunk_w(w, m_chunk):
        # [L, K, M] -> [L, M//m_chunk, 128, K//128, m_chunk] matching SBUF lhsT tiles
        Lw, Kw, Mw = w.shape
        a = w.reshape(Lw, Kw // P, P, Mw // m_chunk, m_chunk)
        return np.ascontiguousarray(a.transpose(0, 3, 2, 1, 4)).astype(bf)

    base = {
        "wq": chunk_w(np.asarray(inputs["wq"][:L]), 512),
        "wk": chunk_w(np.asarray(inputs["wk"][:L]), 512),
        "wv": chunk_w(np.asarray(inputs["wv"][:L]), 512),
        "wo": chunk_w(np.asarray(inputs["wo"][:L]), 512),
        "w1": chunk_w(np.asarray(inputs["w1"][:L]), 512),
        "w2": chunk_w(np.asarray(inputs["w2"][:L]), P),
        "bq": np.ascontiguousarray(inputs["bq"][:L]).astype(np.float32),
        "bk": np.ascontiguousarray(inputs["bk"][:L]).astype(np.float32),
        "bo": np.ascontiguousarray(inputs["bo"][:L]).astype(np.float32),
        "b1": np.ascontiguousarray(inputs["b1"][:L]).astype(np.float32),
        "b2": np.ascontiguousarray(inputs["b2"][:L]).astype(np.float32),
        "bv": np.ascontiguousarray(inputs["bv"][:L]).astype(bf),
        "g1": np.ascontiguousarray(inputs["ln1_g"][:L]).astype(np.float32),
        "be1": np.ascontiguousarray(inputs["ln1_b"][:L]).astype(np.float32),
        "g2": np.ascontiguousarray(inputs["ln2_g"][:L]).astype(np.float32),
        "be2": np.ascontiguousarray(inputs["ln2_b"][:L]).astype(np.float32),
    }
    in_maps = []
    for c in range(R):
        m = dict(base)
        m["xT"] = np.ascontiguousarray(x[0, c * SL:(c + 1) * SL, :].T.astype(np.float32))
        in_maps.append(m)
    return in_maps


def run(inputs, n_layers=4, trace=False):
    if n_layers not in _cache:
        _cache[n_layers] = build(n_layers)
    nc = _cache[n_layers]
    in_maps = _prep_inputs(inputs, n_layers)
    res = bass_utils.run_bass_kernel_spmd(nc, in_maps, core_ids=list(range(R)),
                                          trace=trace)
    out = np.empty((1, S, D), dtype=np.float32)
    for c in range(R):
        out[0, c * SL:(c + 1) * SL, :] = res.results[c]["out"].T
    return out, res


def kernel(**inputs) -> np.ndarray:
    out, _ = run(inputs, n_layers=4, trace=False)
    return out
